# revision 16
# baseline (speedup 1.0000x reference)
"""Trainium2 Bass kernel for nn_BaseEBM (EBM inner gradient-descent loop).

Computation (per sample, matching the reference):
    y = y_mean
    repeat 20x:  y <- y - 0.1 * dE/dy
    E(x, y) = W3 @ relu(W2 @ relu(W1 @ relu(W0 @ [x, y] + b0) + b1) + b2) + b3

Distribution: pure data parallel over 8 NeuronCores (131072 samples each).

Device layout: feature-major [128, 512] tiles. Four independent sample
groups of 32 features are block-diagonally packed across the 128
partitions; 512 samples per group per tile -> 2048 samples/tile, 64
tiles/core. All matmuls use block-diagonal weights so one instruction
processes 4 groups at full PE rate (fp32r, 1 cycle/column).

Key algebraic restructurings:
  * The energy head (layer 3 forward) is never computed; W3 is folded into
    the first backward matmul: g1p = blkdiag(W2 * W3^T) @ m2.
  * x never changes across steps, so z0 = W0 @ [x, y] is kept resident in
    PSUM for all 20 steps and updated by accumulating matmuls:
        dz0 = -lr * w0y (w0y . g0) = blkdiag(P) @ g0,  P = -lr w0y w0y^T.
  * y is never materialized during the loop.  Since dz0 = w0y * dy, the
    final y is recovered from the PSUM residue:
        y = (z0_fin[f*] - z0_init[f*]) / w0y[f*] + y_mean
    (f* = argmax |w0y|; the init snapshot cancels the init rounding).
    This frees a PSUM bank per chain, allowing FOUR independent
    tile-chains in the 8 banks - needed because one chain's per-step
    dependency path (~7 us) is ~4x its per-engine work (~1.6 us).
  * Masked backprop uses the fused DVE op (h > 0) * g in one instruction
    (scalar_tensor_tensor with is_gt + mult), so relu masks are never
    materialized for layers 0/1.
  * The transient-PSUM pool is bufs=1 per chain: the slot-reuse deps
    exactly coincide with the data deps (z2 can only start after h1,
    which is when z1's bank frees), so one bank per chain costs nothing.
  * The layer-2 mask m2 is computed two ways on a per-(step, chain)
    pattern chosen to balance ACT vs DVE busy time (~60/40) and to
    stagger the DVE-heavy steps across chains: 3/5 of steps use one
    exact ACT Sign op (s2 = sign(z2+b2) in {-1,0,1}; V@m2 == (V/2)@s2
    + c with c = (V/2)@ones, the c term a rank-1 matmul off a constant
    ones row so it has no input dependency), 2/5 use one DVE is_gt
    straight from PSUM.
  * Extraction is restructured to 2 compute ops + 2 DMAs per tile: the
    host pre-scales x by cfac (L0 weights compensate), the z0->SBUF
    copy carries scale=inv, one strided 4-row gather lands z0[f*] rows
    on the even partitions of an [8,N] tile, and a single all-SBUF
    2x-mode DVE subtract against the input tile yields y on the even
    rows, stored by one strided DMA.  Both extraction DMAs ride the
    idle GpSimd SWDGE queue so the SP queue only carries input loads
    (16 single-row SP DMAs per quad used to serialize ~25us at every
    quad boundary).

Per step per tile: 5-6 matmuls, 2 ACT relus + the m2 op as above, and
2 fused DVE mask-multiplies ((h>0)*g in one scalar_tensor_tensor).

HW-measured scheduling notes (sim was neutral-to-wrong on all of
these): io double-buffering (bufs=2) regresses ~+20%%; continuous
cross-quad chain pipelining regresses ~+10%% (strict-FIFO engines
head-of-line block on far-future ops); a 2-op is_gt+mult tensor_scalar
from PSUM costs ~2.6x the 1-op form, so the {0,2}-mask/shared-half-V
trick loses; PRIME=3 beats 2 and 5.

Session-2 HW tuning (microbench-calibrated: MM f32r 255ns, ACT 699ns,
DVE stt 749ns per [128,512] op -> engine work ~2.3ms vs 3.7ms measured
= stall-bound, not work-bound): SB_BUFS 2->4 cut exec 3.70 -> 2.89 ms
(-22%%) by letting producers run ahead of lagging consumers (convoy
absorption); SB_BUFS=5 overflows SBUF; asymmetric split pools (h@5 +
m/g@3) regress to 3.47; quad-batched input DMA + io bufs=2 (11.9ms),
QPB=4 (3.98), per-body PE HAM warm-up bursts (4.00), and m2-pattern
shifts (neutral) all fail.  tc.For_i carries an all-engine barrier
per iteration; halving the barrier count still regressed.

Session-3 notes: run-to-run noise on identical configs is ~+-10%
(device warms across a session; documented P0 downclock), so only
>10%% effects are resolvable.  QPB=1 neutral; staggered_reset For_i
regresses (3.58); io bufs=2 no longer fits SBUF at SB_BUFS=4; engine
ops cannot stride the partition axis (only DMA can), so extraction
keeps the full-tile ACT copy; the HW ACT tables contain
derivative_relu (exact step -> would remove the c matmul) but mybir
does not expose it.  Unused weights (bf16 V splits, warm) are now
declared conditionally and make_weight_arrays filters to match.
Shipped config: SB_BUFS=4, PRIME=3, QPB=2, M2_PAT=(1,1,1,0,0);
measured 2.89-3.17 ms across runs vs 3.47-3.70 baseline.
"""

import numpy as np

import concourse.bass as bass
import concourse.mybir as mybir
import concourse.tile as tile
from concourse import bacc
from concourse.bass_utils import run_bass_kernel_spmd

F32 = mybir.dt.float32
F32R = mybir.dt.float32r
BF16 = mybir.dt.bfloat16
ALU = mybir.AluOpType
AF = mybir.ActivationFunctionType

B = 1048576
NCORES = 8
BC = B // NCORES           # 131072 samples per core
G = 4                      # sample groups packed across partitions
TILE_N = 512               # samples per group per tile (PSUM bank limit)
SPT = G * TILE_N           # 2048 samples per tile
NT_FULL = BC // SPT        # 64 tiles per core
STEPS = 20
LR = 0.1
W = 32
NCHAINS = 4
SHARED_TMP = False
TMP_BUFS = 3
DYN = True      # hardware For_i loop over tile-quads
FWD_FP32 = False  # true-fp32 forward matmuls (4 cyc/row) -> ~40x less error
IO_BUFS = 1     # double-buffer quad I/O so next quad's DMAs prefetch
EXT_GPSIMD = True  # extraction DMAs on GpSimd SWDGE queue (else SP)
# Layer-2 mask via one exact ACT Sign op on ACT-path steps:
# s2 = sign(z2+b2) in {-1,0,1}; V@m2 == (V/2)@s2 + c with c = (V/2)@ones,
# so sign-steps use half-scaled V weights plus a rank-1 constant matmul.
# per-(step, chain) pattern: 1 -> ACT Sign, 0 -> DVE is_gt from PSUM.
# 3/5 on ACT balances ACT (h0,h1,m2) vs DVE (g1,g0,m2) busy; the 2*c
# stagger spreads DVE-heavy steps across chains in time.
M2_SIGN = True
V_F32R = True   # single f32r V matmul for g1p (vs bf16 hi+lo split)
C_FIRST = True  # dependency-free c matmul opens the g1p PSUM group
M2_PAT = (1, 1, 1, 0, 0)
PRIME = 3   # chain emission phase offset (yields)
SB_BUFS = 4 # per-chain SBUF pool depth (HW-tuned: 2->4 is -22% exec)
QPB = 2     # quads per For_i body

# --- experiment overrides (defaults above = shipped config) ---
import os as _os
if _os.environ.get("K_M2PAT"):
    M2_PAT = tuple(int(c) for c in _os.environ["K_M2PAT"])
if _os.environ.get("K_PRIME"):
    PRIME = int(_os.environ["K_PRIME"])
if _os.environ.get("K_QPB"):
    QPB = int(_os.environ["K_QPB"])
if _os.environ.get("K_SBBUFS"):
    SB_BUFS = int(_os.environ["K_SBBUFS"])
IO_BIG = bool(int(_os.environ.get("K_IOBIG", "0")))  # one input DMA per quad
if _os.environ.get("K_IOBUFS"):
    IO_BUFS = int(_os.environ["K_IOBUFS"])
STAGGER = bool(int(_os.environ.get("K_STAGGER", "0")))  # For_i staggered_reset
WARM = int(_os.environ.get("K_WARM", "0"))  # PE HAM warm-up MMs per loop body
if _os.environ.get("K_SHTMP"):
    SHARED_TMP = True
    TMP_BUFS = int(_os.environ["K_SHTMP"])
SBH = int(_os.environ.get("K_SBH", "5"))  # h0/h1/g1/g0 pool depth
SBG = int(_os.environ.get("K_SBG", "0"))  # split pools: m2/g1/g0 depth
EXT2 = bool(int(_os.environ.get("K_EXT2", "0")))  # strided ACT ext, no gather
IP_BUFS = int(_os.environ.get("K_IPBUFS", "1"))  # input-tile prefetch depth
M2IO = bool(int(_os.environ.get("K_M2IO", "1")))  # m2 in shallow io pool



def _emit_tile_chain(nc, t, c, dram, wt, sb, ptmp, pz0, io, fstar, inv, cfac,
                     inpq=None):
    sb, sbg = sb if isinstance(sb, tuple) else (sb, sb)
    """Generator emitting one packed tile's program; yields between steps
    so NCHAINS chains interleave in emission (and thus in the static
    per-engine schedules)."""
    _dyn = not isinstance(t, int)
    dst = dram["yout"][bass.ds(t, 1)][0, c] if _dyn else dram["yout"][t][c]
    if IO_BIG:
        # column slice of the quad-level input tile loaded by emit_quad
        inp_ap = inpq[:, c * TILE_N:(c + 1) * TILE_N]
    else:
        src = dram["inp0"][bass.ds(t, 1)][0, c] if _dyn else dram["inp0"][t][c]
        ipool = io if isinstance(io, tile.TilePool) else io[1]
        inp = ipool.tile([2 * G, TILE_N], F32 if FWD_FP32 else F32R,
                         tag=f"inp{c}", name=f"inp_{c}")
        nc.sync.dma_start(out=inp[:], in_=src)
        inp_ap = inp[:]
        io = io if isinstance(io, tile.TilePool) else io[0]

    z0 = pz0.tile([128, TILE_N], F32, tag="z0", name=f"z0_{c}")
    # z0 = blkdiag(W0) @ [x; y_mean]   (no bias; ACT adds b0 every step)
    nc.tensor.matmul(
        z0[:], wt["L0"][:], inp_ap,
        start=True, stop=False, skip_group_check=True,
    )
    yield

    for s in range(STEPS):
        HDT = F32 if FWD_FP32 else F32R
        h0 = sb.tile([128, TILE_N], HDT, tag="h0", name=f"h0_{c}")
        nc.scalar.activation(h0[:], z0[:], AF.Relu, bias=wt["b0"][:])
        yield
        z1 = ptmp.tile([128, TILE_N], F32, tag="tmp", name=f"z1_{c}")
        nc.tensor.matmul(
            z1[:], wt["Lz1"][:], h0[:],
            start=True, stop=True, skip_group_check=True,
        )
        yield
        h1 = sb.tile([128, TILE_N], HDT, tag="h1", name=f"h1_{c}")
        nc.scalar.activation(h1[:], z1[:], AF.Relu, bias=wt["b1"][:])
        yield
        z2 = ptmp.tile([128, TILE_N], F32, tag="tmp", name=f"z2_{c}")
        nc.tensor.matmul(
            z2[:], wt["Lz2"][:], h1[:],
            start=True, stop=True, skip_group_check=True,
        )
        yield
        _m2pool = io if M2IO else sbg
        m2 = _m2pool.tile([128, TILE_N], F32R if V_F32R else BF16,
                          tag=f"m2_{c}" if M2IO else "m2", name=f"m2_{c}")
        # GpSimd tensor_scalar measured ~8us/op on HW - never use it.
        # The mask is exact on both paths: ACT Sign gives {-1,0,1} (the
        # +-1 scale is absorbed by half-V weights + the rank-1 c matmul),
        # DVE is_gt from PSUM gives {0,1}.
        if M2_SIGN:
            act_path = M2_PAT[(s + 2 * c) % len(M2_PAT)] == 1
        else:
            act_path = s % 3 != 2
        g1p = ptmp.tile([128, TILE_N], F32, tag="tmp", name=f"g1p_{c}")
        if act_path and M2_SIGN:
            nc.scalar.activation(m2[:], z2[:], AF.Sign, bias=wt["b2"][:])
            yield
            # g1p = blk(V/2) @ s2 + c (x) ones; with C_FIRST the
            # dependency-free c matmul opens the group so PE overlaps
            # the ACT sign op instead of waiting for m2
            if C_FIRST:
                nc.tensor.matmul(
                    g1p[:], wt["Lc"][:], wt["ones"][:],
                    start=True, stop=False, skip_group_check=True,
                )
            if V_F32R:
                nc.tensor.matmul(
                    g1p[:], wt["Lv2r"][:], m2[:],
                    start=not C_FIRST, stop=C_FIRST, skip_group_check=True,
                )
            else:
                nc.tensor.matmul(
                    g1p[:], wt["Lg1h2"][:], m2[:],
                    start=not C_FIRST, stop=False, skip_group_check=True,
                )
                nc.tensor.matmul(
                    g1p[:], wt["Lg1l2"][:], m2[:],
                    start=False, stop=C_FIRST, skip_group_check=True,
                )
            if not C_FIRST:
                nc.tensor.matmul(
                    g1p[:], wt["Lc"][:], wt["ones"][:],
                    start=False, stop=True, skip_group_check=True,
                )
        else:
            if act_path:
                h2 = sbg.tile([128, TILE_N], BF16, tag="h2", name=f"h2_{c}")
                nc.scalar.activation(h2[:], z2[:], AF.Relu, bias=wt["b2"][:])
                yield
                nc.vector.tensor_scalar(m2[:], h2[:], 0.0, None, ALU.is_gt)
            else:
                nc.vector.tensor_scalar(m2[:], z2[:], wt["nb2"][:], None,
                                        ALU.is_gt)
            yield
            # g1p = blkdiag(W2 * W3^T) @ m2: one f32r matmul (~11-bit
            # weights) or split-bf16 hi+lo (~16-bit effective).
            if V_F32R:
                nc.tensor.matmul(
                    g1p[:], wt["Lvr"][:], m2[:],
                    start=True, stop=True, skip_group_check=True,
                )
            else:
                nc.tensor.matmul(
                    g1p[:], wt["Lg1h"][:], m2[:],
                    start=True, stop=False, skip_group_check=True,
                )
                nc.tensor.matmul(
                    g1p[:], wt["Lg1l"][:], m2[:],
                    start=False, stop=True, skip_group_check=True,
                )
        yield
        g1 = sbg.tile([128, TILE_N], F32R, tag="g1", name=f"g1_{c}")
        nc.vector.scalar_tensor_tensor(
            g1[:], h1[:], 0.0, g1p[:], op0=ALU.is_gt, op1=ALU.mult
        )
        yield
        g0p = ptmp.tile([128, TILE_N], F32, tag="tmp", name=f"g0p_{c}")
        nc.tensor.matmul(
            g0p[:], wt["Lg0"][:], g1[:],
            start=True, stop=True, skip_group_check=True,
        )
        yield
        g0 = sbg.tile([128, TILE_N], F32R, tag="g0", name=f"g0_{c}")
        nc.vector.scalar_tensor_tensor(
            g0[:], h0[:], 0.0, g0p[:], op0=ALU.is_gt, op1=ALU.mult
        )
        yield
        # z0 += blkdiag(P) @ g0  == w0y (x) dy for this step
        nc.tensor.matmul(
            z0[:], wt["LP"][:], g0[:],
            start=False, stop=(s == STEPS - 1), skip_group_check=True,
        )
        yield

    # y = z0_fin[f*] * inv - cfac * x   (inv = 1/W0[f*,1], cfac =
    # W0[f*,0]*inv; y_mean cancels: z0_init[f*] includes W0[f*,1]*y_mean).
    # The host pre-scales x by cfac in inp0 (L0 weights compensate), so
    # extraction is: scale-by-inv copy (ACT), one strided 4-row gather DMA
    # into the even partitions of zf8, one all-SBUF 2x-mode DVE subtract
    # against inp, one strided store DMA.  The two extraction DMAs ride
    # the idle GpSimd SWDGE queue, keeping SP free for input prefetch.
    _xdma = nc.gpsimd.dma_start if EXT_GPSIMD else nc.sync.dma_start
    zf8 = io.tile([2 * G, TILE_N], F32, tag=f"zf{c}", name=f"zf_{c}")
    if EXT2:
        # write z0[f*] rows (scaled by inv) straight onto the even
        # partitions of zf8 -- no full-tile copy, no gather DMA
        nc.scalar.activation(zf8[0:2 * G:2, :],
                             z0[fstar:fstar + 32 * (G - 1) + 1:32, :],
                             AF.Copy, scale=inv)
    else:
        zblk = io.tile([128, TILE_N], F32, tag=f"zb{c}", name=f"zb_{c}")
        nc.scalar.activation(zblk[:], z0[:], AF.Copy, scale=inv)
        _xdma(out=zf8[0:2 * G:2, :],
              in_=zblk[fstar:fstar + 32 * (G - 1) + 1:32, :])
    yo8 = io.tile([2 * G, TILE_N], F32, tag=f"yo{c}", name=f"yo_{c}")
    # yo8 = zf8 - inp  (even rows: z0[f*]*inv - cfac*x = y; odd rows junk)
    nc.vector.scalar_tensor_tensor(yo8[:], zf8[:], 1.0, inp_ap.bitcast(F32),
                                   op0=ALU.mult, op1=ALU.subtract)
    _xdma(out=dst, in_=yo8[0:2 * G:2, :])
    yield


def build(nt=NT_FULL, fstar=0, inv=1.0, cfac=1.0, reps=1, dyn=None):
    """Build + compile the per-core Bass program for nt packed tiles."""
    nc = bacc.Bacc("TRN2", target_bir_lowering=False, debug=False,
                   num_devices=NCORES)

    ntq = nt // NCHAINS
    inp0_shape = ([ntq, 2 * G, NCHAINS * TILE_N] if IO_BIG else
                  [ntq, NCHAINS, 2 * G, TILE_N])
    dram = {
        "inp0": nc.dram_tensor("inp0", inp0_shape,
                               F32 if FWD_FP32 else F32R,
                               kind="ExternalInput").ap(),
        "yout": nc.dram_tensor("yout", [ntq, NCHAINS, G, TILE_N], F32,
                               kind="ExternalOutput").ap(),
    }
    wspec = {
        "L0": [2 * G, 128],
        "Lz1": [128, 128], "Lz2": [128, 128],
        "Lg0": [128, 128],
        "Lvr": [128, 128], "Lv2r": [128, 128],
        "Lc": [1, 128], "ones": [1, TILE_N],
        "LP": [128, 128],
        "b0": [128, 1], "b1": [128, 1], "b2": [128, 1], "nb2": [128, 1],
    }
    if not V_F32R:
        wspec.update({"Lg1h": [128, 128], "Lg1l": [128, 128],
                      "Lg1h2": [128, 128], "Lg1l2": [128, 128]})
    if WARM:
        wspec["warm"] = [128, TILE_N]
    global _ACTIVE_WKEYS
    _ACTIVE_WKEYS = {f"w_{k}" for k in wspec}
    fwd = F32 if FWD_FP32 else F32R
    wdtype = {k: (F32 if k in ("b0", "b1", "b2", "nb2") else
                  (BF16 if k in ("Lg1h", "Lg1l", "Lg1h2", "Lg1l2") else
                   (fwd if k in ("Lz1", "Lz2", "L0") else F32R)))
              for k in wspec}
    wdtype["warm"] = F32R
    wdram = {k: nc.dram_tensor(f"w_{k}", sh, wdtype[k],
                               kind="ExternalInput").ap()
             for k, sh in wspec.items()}

    with tile.TileContext(nc) as tc:
        import contextlib
        with contextlib.ExitStack() as ctx:
            wp = ctx.enter_context(tc.tile_pool(name="wp", bufs=1))
            io = ctx.enter_context(tc.tile_pool(name="io", bufs=IO_BUFS))
            if IP_BUFS > 1:
                ip = ctx.enter_context(tc.tile_pool(name="ip", bufs=IP_BUFS))
                io = (io, ip)
            sbs = [ctx.enter_context(tc.tile_pool(name=f"sb{c}",
                                                   bufs=(SBH or SB_BUFS)))
                   for c in range(NCHAINS)]
            sbgs = (
                [ctx.enter_context(tc.tile_pool(name=f"sg{c}", bufs=SBG))
                 for c in range(NCHAINS)] if SBG else sbs)
            if SHARED_TMP:
                pt = ctx.enter_context(
                    tc.tile_pool(name="pt", bufs=TMP_BUFS, space="PSUM"))
                ptmps = [pt] * NCHAINS
            else:
                ptmps = [ctx.enter_context(
                    tc.tile_pool(name=f"pt{c}", bufs=1, space="PSUM"))
                    for c in range(NCHAINS)]
            pz0s = [ctx.enter_context(
                tc.tile_pool(name=f"pz{c}", bufs=1, space="PSUM"))
                for c in range(NCHAINS)]

            wt = {}
            for k, sh in wspec.items():
                wt[k] = wp.tile(sh, wdtype[k], tag=f"w_{k}", name=f"wt_{k}")
                nc.sync.dma_start(out=wt[k][:], in_=wdram[k][:])

            assert nt % NCHAINS == 0

            def emit_warmup():
                # Dense b2b dummy matmuls to push the PE HAM activity
                # window over the un-throttle threshold (2.4 GHz) after
                # each all-engine barrier re-idles it.
                if not WARM:
                    return
                wz = pz0s[0].tile([128, TILE_N], F32, tag="z0",
                                  name="warmz")
                for _ in range(WARM):
                    nc.tensor.matmul(
                        wz[:], wt["Lz1"][:], wt["warm"][:],
                        start=True, stop=True, skip_group_check=True,
                    )

            def emit_quad(tq):
                inpq = None
                if IO_BIG:
                    _dyn = not isinstance(tq, int)
                    srcq = (dram["inp0"][bass.ds(tq, 1)][0] if _dyn
                            else dram["inp0"][tq])
                    inpq = io.tile([2 * G, NCHAINS * TILE_N],
                                   F32 if FWD_FP32 else F32R,
                                   tag="inpq", name="inpq")
                    nc.sync.dma_start(out=inpq[:], in_=srcq)
                    inpq = inpq[:]
                chains = [
                    _emit_tile_chain(nc, tq, c, dram, wt,
                                     (sbs[c], sbgs[c]), ptmps[c], pz0s[c], io,
                                     fstar, inv, cfac, inpq=inpq)
                    for c in range(NCHAINS)
                ]
                # phase-offset the chains by ~1/NCHAINS of a step so no
                # engine sees two dependent ops of one chain back-to-back
                for c, ch in enumerate(chains):
                    for _ in range(c * PRIME):
                        next(ch)
                alive = list(chains)
                while alive:
                    for ch in list(alive):
                        try:
                            next(ch)
                        except StopIteration:
                            alive.remove(ch)

            use_dyn = DYN if dyn is None else dyn
            if use_dyn:
                def body():
                    assert ntq % QPB == 0
                    with tc.For_i(0, ntq // QPB, 1,
                                  hint_engines=(mybir.EngineType.PE,),
                                  staggered_reset=STAGGER) as iv:
                        ivq = iv * QPB
                        emit_warmup()
                        for j in range(QPB):
                            emit_quad(ivq + j)
                if reps > 1:
                    with tc.For_i(0, reps, 1):
                        body()
                else:
                    body()
            else:
                for tq in range(ntq):
                    emit_quad(tq)

    nc.compile()
    return nc


_ACTIVE_WKEYS = None


def make_weight_arrays(W0, b0, W1, b1, W2, b2, W3, b3, y_mean):
    """Host-side constant construction (all small).  Returns only the
    weights declared by the last build() (or all, if build not yet run)."""
    import ml_dtypes
    eye = np.eye(G, dtype=np.float32)
    blk = lambda A: np.kron(eye, A.astype(np.float32))
    fstar, inv, cfac = extraction_consts(W0)
    w0y = W0[:, 1].astype(np.float32)
    P = (-LR) * np.outer(w0y, w0y)
    V = blk(W2 * W3[0][:, None])
    Vh = V.astype(ml_dtypes.bfloat16)
    Vl = (V - Vh.astype(np.float32)).astype(ml_dtypes.bfloat16)
    V2 = 0.5 * V
    V2h = V2.astype(ml_dtypes.bfloat16)
    V2l = (V2 - V2h.astype(np.float32)).astype(ml_dtypes.bfloat16)
    # x arrives pre-scaled by cfac in inp0; fold 1/cfac into L0's x column
    assert cfac != 0.0
    W0m = W0.astype(np.float32).copy()
    W0m[:, 0] /= np.float32(cfac)
    out = {
        "w_L0": blk(W0m.T),                      # [8, 128]
        "w_Lz1": blk(W1.T),                      # [128, 128]
        "w_Lz2": blk(W2.T),                      # [128, 128]
        "w_Lg1h": Vh, "w_Lg1l": Vl,              # bf16 split of V
        "w_Lg1h2": V2h, "w_Lg1l2": V2l,          # bf16 split of V/2
        "w_Lvr": V, "w_Lv2r": V2,                # f32r single-matmul V
        "w_Lc": (0.5 * V.sum(axis=0))[None, :].astype(np.float32),
        "w_ones": np.ones((1, TILE_N), np.float32),
        "w_Lg0": blk(W1),                        # [128, 128]
        "w_LP": blk(P),                          # [128, 128]
        "w_warm": np.zeros((128, TILE_N), np.float32),
        "w_b0": np.tile(b0.astype(np.float32), G)[:, None],
        "w_b1": np.tile(b1.astype(np.float32), G)[:, None],
        "w_b2": np.tile(b2.astype(np.float32), G)[:, None],
        "w_nb2": np.tile(-b2.astype(np.float32), G)[:, None],
    }
    if _ACTIVE_WKEYS is not None:
        out = {k: v for k, v in out.items() if k in _ACTIVE_WKEYS}
    return out


def extraction_consts(W0):
    W0 = np.asarray(W0, np.float32)
    fstar = int(np.argmax(np.abs(W0[:, 1])))
    inv = float(1.0 / W0[fstar, 1])
    cfac = float(W0[fstar, 0] * inv)
    return fstar, inv, cfac


def make_core_inputs(x, y_mean, nt=NT_FULL, cfac=1.0):
    """Per-core input tiles: [nt, 8, 512] with cfac*x on even rows, y_mean
    on odd rows (L0 weights divide the x column by cfac to compensate;
    the extraction subtracts inp directly).  Returns NCORES arrays."""
    xs = np.ascontiguousarray(
        np.asarray(x, np.float32).reshape(NCORES, nt, G, TILE_N))
    ym = np.float32(np.asarray(y_mean).reshape(-1)[0])
    maps = []
    for c in range(NCORES):
        inp0 = np.empty((nt, 2 * G, TILE_N), dtype=np.float32)
        inp0[:, 0::2, :] = np.float32(cfac) * xs[c]
        inp0[:, 1::2, :] = ym
        inp0 = inp0.reshape(nt // NCHAINS, NCHAINS, 2 * G, TILE_N)
        if IO_BIG:
            # [ntq, 2G, NCHAINS*TILE_N] with chain-blocked columns
            inp0 = np.ascontiguousarray(inp0.transpose(0, 2, 1, 3)).reshape(
                nt // NCHAINS, 2 * G, NCHAINS * TILE_N)
        maps.append(inp0)
    return maps


_NC_CACHE = {}


def get_nc(nt, fstar, inv, cfac):
    key = (nt, fstar, round(inv, 9), round(cfac, 9))
    if key not in _NC_CACHE:
        _NC_CACHE[key] = build(nt, fstar, inv, cfac)
    return _NC_CACHE[key]


def kernel(x, W0, b0, W1, b1, W2, b2, W3, b3, y_mean):
    x = np.asarray(x, dtype=np.float32)
    fstar, inv, cfac = extraction_consts(W0)
    nc = get_nc(NT_FULL, fstar, inv, cfac)

    warr = make_weight_arrays(
        np.asarray(W0), np.asarray(b0), np.asarray(W1), np.asarray(b1),
        np.asarray(W2), np.asarray(b2), np.asarray(W3), np.asarray(b3),
        np.asarray(y_mean))
    inp0s = make_core_inputs(x, np.asarray(y_mean), NT_FULL, cfac)
    in_maps = [{"inp0": inp0s[c], **warr} for c in range(NCORES)]

    res = run_bass_kernel_spmd(nc, in_maps, list(range(NCORES)))
    youts = [res.results[c]["yout"].reshape(BC) for c in range(NCORES)]
    return np.concatenate(youts).reshape(B, 1).astype(np.float32)

